# revision 23
# baseline (speedup 1.0000x reference)
"""Mixtral-style sparse MoE block on 8 Trainium2 NeuronCores.

Strategy: expert-parallel with cross-expert load balancing. The router
(tiny: T x H @ H x E) runs on the host as part of input sharding; the heavy
FFN compute (top-2 of 8 experts = ~720 GFLOP) runs on the cores as bf16
matmuls (78.6 TF/s peak per core). The host applies the renormalized top-2
combine weights and scatter-adds the per-expert outputs back into the full
[T, H] output.

Per-bin kernel math (W = bin token capacity):
    h  = silu(x @ w1) * (x @ w3)    # [W, F] SwiGLU
    y  = h @ w2                     # [W, H]
computed in feature-on-partition layout: activations are [feature, token]
so all three weight matrices are used in their natural [K, M] layout as
matmul stationary operands and the SwiGLU intermediate h lands directly in
the [F-partition, token-free] layout that the down-projection consumes.

Each core runs P weight-stream PASSES. A pass streams one expert's
w1/w3/w2 from HBM exactly once (the per-pass weight tensors are part of the
core's inputs, so WHICH expert a (core, pass) bin serves is a host-side
choice) and applies them to every 512-column PSUM chunk of the pass, so
weight DMA is amortized over the full pass width and stays far from the
358 GB/s/core roofline.

Load balancing: with one expert pinned per core, the padded capacity C is
set by the most-loaded expert. Instead, passes are built as (u+d, u) width
pairs: a heavy expert takes a donor's bin in the wide pass and returns its
bin in the narrow pass (+d tokens for it, -d for the donor), which lets C
approach mean expert load instead of max (seed-0 routing: 4112 vs 4272).

Precision: all matmul operands bf16 (PSUM accumulation is fp32). Measured
L2 rel err 4.1e-3 vs the fp32 oracle (gate: 2e-2). fp8/DoubleRow was
evaluated and rejected: e4m3 quantization of Gaussian activations is 2.7%
RMS and the end-to-end error measures 3.8e-2+, over the gate.

DMA detail: single dma_start calls serialize on one of 16 ~24 GB/s DMA
engines, so weight/output tiles are split into 2-4 pieces to parallelize
across engines (a whole [128, F] bf16 tile would have ~40 us latency).
"""

import numpy as np

H = 1024        # hidden dim
F = 3584        # FFN dim
E = 8           # experts == cores
NT = 512        # chunk width (psum bank = 512 fp32)
WMAX = 1216     # max pass width (SBUF-limited: 28 h-tiles of [128, WMAX] bf16)
WMIN = 512      # min pass width (keeps every pass compute-bound vs 22MB stream)
GRAN = 16       # token-capacity granularity
KH = H // 128   # 8 k-tiles over hidden
MF = F // 128   # 28 m-tiles over ffn
MH = H // 128   # 8 m-tiles over hidden (down-proj output)

_compile_cache = {}
_last_result = None  # BassKernelResults of the most recent run (for profiling)


def _rup(v, g=GRAN):
    return -(-int(v) // g) * g


def _uniform_plan(max_cnt):
    """One expert per core for all passes; widths cover roundup(max_cnt)."""
    c = _rup(max(max_cnt, GRAN))
    n_full, tail = divmod(c, NT)
    widths = []
    for _ in range(n_full // 2):
        widths.append(2 * NT)
    if n_full % 2:
        widths.append(NT)
    if tail:
        if widths and widths[-1] + tail <= WMAX:
            widths[-1] += tail
        else:
            widths.append(tail)
    bins = [list(range(E)) for _ in widths]
    return tuple(widths), bins


def _balanced_plan(cnts):
    """Pair-swap packing: capacity C below roundup(max cnt) when feasible.

    Returns (widths, bins) or None. bins[p][c] = expert id served by core
    c in pass p. Every expert holds exactly one bin per pass, except that
    pair-swaps exchange a (wide, narrow) bin pair between a heavy expert
    and a donor.
    """
    cu = _rup(max(cnts))
    cl = _rup(sum(cnts) / len(cnts))
    for c in range(cl, cu, GRAN):
        heavy = sorted(
            ((cnt - c, e) for e, cnt in enumerate(cnts) if cnt > c), reverse=True
        )
        light = sorted(
            ((c - cnt, e) for e, cnt in enumerate(cnts) if cnt <= c), reverse=True
        )
        if not heavy or len(heavy) > len(light):
            continue
        deltas = [_rup(d) for d, _ in heavy]
        if any(dl > light[i][0] for i, dl in enumerate(deltas)):
            continue
        npairs = len(heavy)
        nfill = max(0, -(-c // WMAX) - 2 * npairs, 4 - 2 * npairs)
        while True:
            p = 2 * npairs + nfill
            rem = c - sum(deltas)
            base = (rem // p) // GRAN * GRAN
            if base + max(deltas) <= WMAX and base >= WMIN:
                break
            nfill += 1
            if p > 12:
                break
        if base + max(deltas) > WMAX or base < WMIN:
            continue
        # widths: pair i -> (u_i + d_i, u_i), then fillers; distribute the
        # gran remainder over fillers (or a pair's u in 2*GRAN steps)
        us = [base] * npairs
        fills = [base] * nfill
        left = rem - base * p
        i = 0
        while left >= GRAN and nfill:
            fills[i % nfill] += GRAN
            left -= GRAN
            i += 1
        while left >= 2 * GRAN:
            us[i % npairs] += GRAN
            left -= 2 * GRAN
            i += 1
        if left:
            continue
        widths = []
        for i in range(npairs):
            widths += [us[i] + deltas[i], us[i]]
        widths += fills
        if any(w > WMAX or w < GRAN for w in widths):
            continue
        bins = [list(range(E)) for _ in widths]
        for i, ((_, h), (_, d)) in enumerate(zip(heavy, light)):
            bins[2 * i][d] = h      # heavy takes donor's wide-pass bin
            bins[2 * i + 1][h] = d  # donor takes heavy's narrow-pass bin
        # verify coverage
        cap = [0] * E
        for p_, row in enumerate(bins):
            for e in row:
                cap[e] += widths[p_]
        if all(cap[e] >= cnts[e] for e in range(E)):
            return tuple(widths), bins
    return None


def _chunks_of(width):
    n_full, tail = divmod(width, NT)
    ch = [NT] * n_full
    if tail:
        ch.append(tail)
    return tuple(ch)


def _build(widths):
    """Build + compile the per-core Bass program for the given pass widths."""
    import concourse.bass as bass
    import concourse.mybir as mybir
    import concourse.tile as tile
    from concourse import bacc

    P = len(widths)
    C = sum(widths)
    wmax = max(widths)
    f32 = mybir.dt.float32
    bf16 = mybir.dt.bfloat16
    ts = bass.ts

    nc = bacc.Bacc("TRN2", target_bir_lowering=False, debug=False, num_devices=E)

    xT = nc.dram_tensor("xT", [H, C], bf16, kind="ExternalInput").ap()
    w1s = nc.dram_tensor("w1s", [P, MF, 128, H], bf16, kind="ExternalInput").ap()
    w3s = nc.dram_tensor("w3s", [P, MF, 128, H], bf16, kind="ExternalInput").ap()
    w2s = nc.dram_tensor("w2s", [P, MH, 128, F], bf16, kind="ExternalInput").ap()
    yT = nc.dram_tensor("yT", [H, C], f32, kind="ExternalOutput").ap()

    xT_r = xT.rearrange("(k p) t -> k p t", p=128)
    yT_r = yT.rearrange("(m p) t -> m p t", p=128)

    with tile.TileContext(nc, trace_sim=False) as tc:
        with (
            tc.tile_pool(name="xp", bufs=2) as xp,
            tc.tile_pool(name="w1p", bufs=12) as w1p,
            tc.tile_pool(name="w3p", bufs=12) as w3p,
            tc.tile_pool(name="w2p", bufs=4) as w2p,
            tc.tile_pool(name="hp", bufs=MF + 2) as hp,
            tc.tile_pool(name="hsp", bufs=4) as hsp,
            tc.tile_pool(name="yp", bufs=4) as yp,
            tc.tile_pool(name="ps1p", bufs=2, space="PSUM") as ps1p,
            tc.tile_pool(name="ps3p", bufs=2, space="PSUM") as ps3p,
            tc.tile_pool(name="psyp", bufs=4, space="PSUM") as psyp,
        ):
            off = 0
            for pi, width in enumerate(widths):
                chunks = []
                coff = 0
                for cw in _chunks_of(width):
                    chunks.append((coff, cw))
                    coff += cw
                nup = 1
                if pi == 0 and chunks[0][1] == NT:
                    # small starter chunk: its x lands in ~1.4us (vs 5.5us
                    # for a 512-wide load), so the kernel-start ramp shrinks
                    chunks = [(0, 128), (128, NT - 128)] + chunks[1:]
                    nup = 2

                xt = xp.tile([128, KH, wmax], bf16)

                def load_x(chunk_list):
                    for coff, cw in chunk_list:
                        for k in range(KH):
                            nc.sync.dma_start(
                                xt[:, k, coff : coff + cw],
                                xT_r[k, :, off + coff : off + coff + cw],
                            )

                # chunk 0's x first so its matmuls can start immediately;
                # the other chunks' x is issued after the first weight tiles
                # (DMAs round-robin across engines; issue order sets the
                # critical path at the pass ramp)
                load_x(chunks[:nup])

                h_tiles = []
                for m in range(MF):
                    # split weight-tile loads so each lands on several DMA
                    # engines in parallel (single-engine rate is ~24 GB/s)
                    w1t = w1p.tile([128, H], bf16)
                    for ho in range(2):
                        nc.sync.dma_start(
                            w1t[:, ts(ho, 512)], w1s[pi, m, :, ts(ho, 512)]
                        )
                    w3t = w3p.tile([128, H], bf16)
                    for ho in range(2):
                        nc.sync.dma_start(
                            w3t[:, ts(ho, 512)], w3s[pi, m, :, ts(ho, 512)]
                        )
                    if m == 0:
                        load_x(chunks[nup:])

                    ht = hp.tile([128, wmax], bf16)
                    for coff, cw in chunks:
                        ps1 = ps1p.tile([128, NT], f32)
                        for k in range(KH):
                            nc.tensor.matmul(
                                ps1[:, :cw],
                                w1t[:, ts(k, 128)],
                                xt[:, k, coff : coff + cw],
                                start=(k == 0),
                                stop=(k == KH - 1),
                            )
                        ps3 = ps3p.tile([128, NT], f32)
                        for k in range(KH):
                            nc.tensor.matmul(
                                ps3[:, :cw],
                                w3t[:, ts(k, 128)],
                                xt[:, k, coff : coff + cw],
                                start=(k == 0),
                                stop=(k == KH - 1),
                            )
                        hs = hsp.tile([128, NT], f32)
                        nc.scalar.activation(
                            hs[:, :cw], ps1[:, :cw],
                            mybir.ActivationFunctionType.Silu,
                        )
                        nc.vector.tensor_mul(
                            ht[:, coff : coff + cw], hs[:, :cw], ps3[:, :cw]
                        )
                    h_tiles.append(ht)

                for mh in range(MH):
                    w2t = w2p.tile([128, F], bf16)
                    for fo in range(4):
                        nc.sync.dma_start(
                            w2t[:, ts(fo, 896)], w2s[pi, mh, :, ts(fo, 896)]
                        )
                    for coff, cw in chunks:
                        psy = psyp.tile([128, NT], f32)
                        for kf in range(MF):
                            nc.tensor.matmul(
                                psy[:, :cw],
                                w2t[:, ts(kf, 128)],
                                h_tiles[kf][:, coff : coff + cw],
                                start=(kf == 0),
                                stop=(kf == MF - 1),
                            )
                        yt = yp.tile([128, NT], f32)
                        nc.vector.tensor_copy(yt[:, :cw], psy[:, :cw])
                        h0 = cw // 2
                        nc.sync.dma_start(
                            yT_r[mh, :, off + coff : off + coff + h0],
                            yt[:, :h0],
                        )
                        nc.sync.dma_start(
                            yT_r[mh, :, off + coff + h0 : off + coff + cw],
                            yt[:, h0:cw],
                        )
                off += width

    nc.compile()
    return nc


def _route(x, gate_w, gate_b):
    """Host router: top-2 expert ids + renormalized combine weights."""
    logits = x.astype(np.float32) @ gate_w.astype(np.float32).T + gate_b.astype(
        np.float32
    )
    # top-2 by prob == top-2 by logit (softmax is monotonic); stable sort
    # matches jax.lax.top_k's lower-index-first tie-breaking.
    top2 = np.argsort(-logits, axis=-1, kind="stable")[:, :2]
    l2 = np.take_along_axis(logits, top2, axis=1)
    e2 = np.exp(l2 - l2.max(axis=1, keepdims=True))
    wts = e2 / e2.sum(axis=1, keepdims=True)
    return top2, wts.astype(np.float32)


def kernel(x, gate_w, gate_b, w1, w3, w2):
    import ml_dtypes
    from concourse.bass_utils import run_bass_kernel_spmd

    bf16 = ml_dtypes.bfloat16
    x = np.asarray(x, dtype=np.float32)
    T = x.shape[0]
    top2, wts = _route(x, np.asarray(gate_w), np.asarray(gate_b))

    idx_list, scale_list = [], []
    for e in range(E):
        sel = top2 == e                      # [T, 2] bool
        tok = np.nonzero(sel.any(axis=1))[0]
        idx_list.append(tok)
        # each token picks an expert at most once, so this take is unique
        which = sel[tok, 1].astype(np.int64)  # 0 if slot0, 1 if slot1
        scale_list.append(wts[tok, which])

    cnts = [len(i) for i in idx_list]
    plan = _balanced_plan(cnts)
    if plan is None:
        plan = _uniform_plan(max(cnts))
    widths, bins = plan
    P = len(widths)
    C = sum(widths)

    nc = _compile_cache.get(widths)
    if nc is None:
        nc = _build(widths)
        _compile_cache[widths] = nc

    # split each expert's token list over its bins, in (pass, core) order
    offs = np.cumsum([0] + list(widths))
    bin_tok = {}    # (core, pass) -> token index array
    used = [0] * E
    for p in range(P):
        for ci in range(E):
            e = bins[p][ci]
            take = min(widths[p], cnts[e] - used[e])
            bin_tok[(ci, p)] = idx_list[e][used[e] : used[e] + take]
            used[e] += take
    assert all(used[e] == cnts[e] for e in range(E)), (used, cnts)

    # per-expert bf16 weight blocks in matmul tile layout
    # W[k*128+p, m*128+c] -> [m, p, k*128+c]: 2KB-contiguous lhsT tiles
    w1 = np.asarray(w1, dtype=np.float32)
    w3 = np.asarray(w3, dtype=np.float32)
    w2 = np.asarray(w2, dtype=np.float32)
    w1b = [
        np.ascontiguousarray(
            w1[e].reshape(KH, 128, MF, 128).transpose(2, 1, 0, 3).reshape(MF, 128, H)
        ).astype(bf16)
        for e in range(E)
    ]
    w3b = [
        np.ascontiguousarray(
            w3[e].reshape(KH, 128, MF, 128).transpose(2, 1, 0, 3).reshape(MF, 128, H)
        ).astype(bf16)
        for e in range(E)
    ]
    w2b = [
        np.ascontiguousarray(
            w2[e].reshape(MF, 128, MH, 128).transpose(2, 1, 0, 3).reshape(MH, 128, F)
        ).astype(bf16)
        for e in range(E)
    ]

    xbf = x.astype(bf16)
    in_maps = []
    for ci in range(E):
        xTe = np.zeros((H, C), bf16)
        for p in range(P):
            tok = bin_tok[(ci, p)]
            if len(tok):
                xTe[:, offs[p] : offs[p] + len(tok)] = xbf[tok].T
        in_maps.append(
            {
                "xT": xTe,
                "w1s": np.stack([w1b[bins[p][ci]] for p in range(P)]),
                "w3s": np.stack([w3b[bins[p][ci]] for p in range(P)]),
                "w2s": np.stack([w2b[bins[p][ci]] for p in range(P)]),
            }
        )

    global _last_result
    res = run_bass_kernel_spmd(nc, in_maps, core_ids=list(range(E)))
    _last_result = res

    out = np.zeros((T, H), np.float32)
    for ci in range(E):
        yTe = res.results[ci]["yT"]
        for p in range(P):
            tok = bin_tok[(ci, p)]
            if len(tok) == 0:
                continue
            e = bins[p][ci]
            pos = np.searchsorted(idx_list[e], tok)
            out[tok] += yTe[:, offs[p] : offs[p] + len(tok)].T * scale_list[e][
                pos, None
            ]
    return out


# revision 24
# speedup vs baseline: 1.0055x; 1.0055x over previous
"""Mixtral-style sparse MoE block on 8 Trainium2 NeuronCores.

Strategy: expert-parallel with cross-expert load balancing. The router
(tiny: T x H @ H x E) runs on the host as part of input sharding; the heavy
FFN compute (top-2 of 8 experts = ~720 GFLOP) runs on the cores as bf16
matmuls (78.6 TF/s peak per core). The host applies the renormalized top-2
combine weights and scatter-adds the per-expert outputs back into the full
[T, H] output.

Per-bin kernel math (W = bin token capacity):
    h  = silu(x @ w1) * (x @ w3)    # [W, F] SwiGLU
    y  = h @ w2                     # [W, H]
computed in feature-on-partition layout: activations are [feature, token]
so all three weight matrices are used in their natural [K, M] layout as
matmul stationary operands and the SwiGLU intermediate h lands directly in
the [F-partition, token-free] layout that the down-projection consumes.

Each core runs P weight-stream PASSES. A pass streams one expert's
w1/w3/w2 from HBM exactly once (the per-pass weight tensors are part of the
core's inputs, so WHICH expert a (core, pass) bin serves is a host-side
choice) and applies them to every 512-column PSUM chunk of the pass, so
weight DMA is amortized over the full pass width and stays far from the
358 GB/s/core roofline.

Load balancing: with one expert pinned per core, the padded capacity C is
set by the most-loaded expert. Instead, passes are built as (u+d, u) width
pairs: a heavy expert takes a donor's bin in the wide pass and returns its
bin in the narrow pass (+d tokens for it, -d for the donor), which lets C
approach mean expert load instead of max (seed-0 routing: 4112 vs 4272).

Precision: all matmul operands bf16 (PSUM accumulation is fp32). Measured
L2 rel err 4.1e-3 vs the fp32 oracle (gate: 2e-2). fp8/DoubleRow was
evaluated and rejected: e4m3 quantization of Gaussian activations is 2.7%
RMS and the end-to-end error measures 3.8e-2+, over the gate.

DMA detail: single dma_start calls serialize on one of 16 ~24 GB/s DMA
engines, so weight/output tiles are split into 2-4 pieces to parallelize
across engines (a whole [128, F] bf16 tile would have ~40 us latency).
"""

import numpy as np

H = 1024        # hidden dim
F = 3584        # FFN dim
E = 8           # experts == cores
NT = 512        # chunk width (psum bank = 512 fp32)
WMAX = 1216     # max pass width (SBUF-limited: 28 h-tiles of [128, WMAX] bf16)
WMIN = 512      # min pass width (keeps every pass compute-bound vs 22MB stream)
GRAN = 16       # token-capacity granularity
KH = H // 128   # 8 k-tiles over hidden
MF = F // 128   # 28 m-tiles over ffn
MH = H // 128   # 8 m-tiles over hidden (down-proj output)

_compile_cache = {}
_last_result = None  # BassKernelResults of the most recent run (for profiling)


def _rup(v, g=GRAN):
    return -(-int(v) // g) * g


def _uniform_plan(max_cnt):
    """One expert per core for all passes; widths cover roundup(max_cnt)."""
    c = _rup(max(max_cnt, GRAN))
    n_full, tail = divmod(c, NT)
    widths = []
    for _ in range(n_full // 2):
        widths.append(2 * NT)
    if n_full % 2:
        widths.append(NT)
    if tail:
        if widths and widths[-1] + tail <= WMAX:
            widths[-1] += tail
        else:
            widths.append(tail)
    bins = [list(range(E)) for _ in widths]
    return tuple(widths), bins


def _balanced_plan(cnts):
    """Pair-swap packing: capacity C below roundup(max cnt) when feasible.

    Returns (widths, bins) or None. bins[p][c] = expert id served by core
    c in pass p. Every expert holds exactly one bin per pass, except that
    pair-swaps exchange a (wide, narrow) bin pair between a heavy expert
    and a donor.
    """
    cu = _rup(max(cnts))
    cl = _rup(sum(cnts) / len(cnts))
    for c in range(cl, cu, GRAN):
        heavy = sorted(
            ((cnt - c, e) for e, cnt in enumerate(cnts) if cnt > c), reverse=True
        )
        light = sorted(
            ((c - cnt, e) for e, cnt in enumerate(cnts) if cnt <= c), reverse=True
        )
        if not heavy or len(heavy) > len(light):
            continue
        deltas = [_rup(d) for d, _ in heavy]
        if any(dl > light[i][0] for i, dl in enumerate(deltas)):
            continue
        npairs = len(heavy)
        nfill = max(0, -(-c // WMAX) - 2 * npairs, 4 - 2 * npairs)
        while True:
            p = 2 * npairs + nfill
            rem = c - sum(deltas)
            base = (rem // p) // GRAN * GRAN
            if base + max(deltas) <= WMAX and base >= WMIN:
                break
            nfill += 1
            if p > 12:
                break
        if base + max(deltas) > WMAX or base < WMIN:
            continue
        # widths: pair i -> (u_i + d_i, u_i), then fillers; distribute the
        # gran remainder over fillers (or a pair's u in 2*GRAN steps)
        us = [base] * npairs
        fills = [base] * nfill
        left = rem - base * p
        i = 0
        while left >= GRAN and nfill:
            fills[i % nfill] += GRAN
            left -= GRAN
            i += 1
        while left >= 2 * GRAN:
            us[i % npairs] += GRAN
            left -= 2 * GRAN
            i += 1
        if left:
            continue
        widths = []
        for i in range(npairs):
            widths += [us[i] + deltas[i], us[i]]
        widths += fills
        if any(w > WMAX or w < GRAN for w in widths):
            continue
        bins = [list(range(E)) for _ in widths]
        for i, ((_, h), (_, d)) in enumerate(zip(heavy, light)):
            bins[2 * i][d] = h      # heavy takes donor's wide-pass bin
            bins[2 * i + 1][h] = d  # donor takes heavy's narrow-pass bin
        # verify coverage
        cap = [0] * E
        for p_, row in enumerate(bins):
            for e in row:
                cap[e] += widths[p_]
        if all(cap[e] >= cnts[e] for e in range(E)):
            return tuple(widths), bins
    return None


def _chunks_of(width):
    n_full, tail = divmod(width, NT)
    ch = [NT] * n_full
    if tail:
        ch.append(tail)
    return tuple(ch)


def _build(widths):
    """Build + compile the per-core Bass program for the given pass widths."""
    import concourse.bass as bass
    import concourse.mybir as mybir
    import concourse.tile as tile
    from concourse import bacc

    P = len(widths)
    C = sum(widths)
    wmax = max(widths)
    f32 = mybir.dt.float32
    bf16 = mybir.dt.bfloat16
    ts = bass.ts

    nc = bacc.Bacc("TRN2", target_bir_lowering=False, debug=False, num_devices=E)

    xT = nc.dram_tensor("xT", [H, C], bf16, kind="ExternalInput").ap()
    w1s = nc.dram_tensor("w1s", [P, MF, 128, H], bf16, kind="ExternalInput").ap()
    w3s = nc.dram_tensor("w3s", [P, MF, 128, H], bf16, kind="ExternalInput").ap()
    w2s = nc.dram_tensor("w2s", [P, MH, 128, F], bf16, kind="ExternalInput").ap()
    yT = nc.dram_tensor("yT", [H, C], f32, kind="ExternalOutput").ap()

    xT_r = xT.rearrange("(k p) t -> k p t", p=128)
    yT_r = yT.rearrange("(m p) t -> m p t", p=128)

    with tile.TileContext(nc, trace_sim=False) as tc:
        with (
            tc.tile_pool(name="xp", bufs=2) as xp,
            tc.tile_pool(name="w1p", bufs=12) as w1p,
            tc.tile_pool(name="w3p", bufs=12) as w3p,
            tc.tile_pool(name="w2p", bufs=4) as w2p,
            tc.tile_pool(name="hp", bufs=MF + 2) as hp,
            tc.tile_pool(name="hsp", bufs=4) as hsp,
            tc.tile_pool(name="yp", bufs=4) as yp,
            tc.tile_pool(name="ps1p", bufs=2, space="PSUM") as ps1p,
            tc.tile_pool(name="ps3p", bufs=2, space="PSUM") as ps3p,
            tc.tile_pool(name="psyp", bufs=4, space="PSUM") as psyp,
        ):
            off = 0
            for pi, width in enumerate(widths):
                chunks = []
                coff = 0
                for cw in _chunks_of(width):
                    chunks.append((coff, cw))
                    coff += cw

                xt = xp.tile([128, KH, wmax], bf16)

                def load_x(chunk_list):
                    for coff, cw in chunk_list:
                        for k in range(KH):
                            nc.sync.dma_start(
                                xt[:, k, coff : coff + cw],
                                xT_r[k, :, off + coff : off + coff + cw],
                            )

                # chunk 0's x first so its matmuls can start immediately;
                # the other chunks' x is issued after the first weight tiles
                # (DMAs round-robin across engines; issue order sets the
                # critical path at the pass ramp)
                load_x(chunks[:1])

                h_tiles = []
                for m in range(MF):
                    # split weight-tile loads so each lands on several DMA
                    # engines in parallel (single-engine rate is ~24 GB/s)
                    w1t = w1p.tile([128, H], bf16)
                    for ho in range(2):
                        nc.sync.dma_start(
                            w1t[:, ts(ho, 512)], w1s[pi, m, :, ts(ho, 512)]
                        )
                    w3t = w3p.tile([128, H], bf16)
                    for ho in range(2):
                        nc.sync.dma_start(
                            w3t[:, ts(ho, 512)], w3s[pi, m, :, ts(ho, 512)]
                        )
                    if m == 0:
                        load_x(chunks[1:])

                    ht = hp.tile([128, wmax], bf16)
                    for coff, cw in chunks:
                        ps1 = ps1p.tile([128, NT], f32)
                        for k in range(KH):
                            nc.tensor.matmul(
                                ps1[:, :cw],
                                w1t[:, ts(k, 128)],
                                xt[:, k, coff : coff + cw],
                                start=(k == 0),
                                stop=(k == KH - 1),
                            )
                        ps3 = ps3p.tile([128, NT], f32)
                        for k in range(KH):
                            nc.tensor.matmul(
                                ps3[:, :cw],
                                w3t[:, ts(k, 128)],
                                xt[:, k, coff : coff + cw],
                                start=(k == 0),
                                stop=(k == KH - 1),
                            )
                        hs = hsp.tile([128, NT], f32)
                        nc.scalar.activation(
                            hs[:, :cw], ps1[:, :cw],
                            mybir.ActivationFunctionType.Silu,
                        )
                        nc.vector.tensor_mul(
                            ht[:, coff : coff + cw], hs[:, :cw], ps3[:, :cw]
                        )
                    h_tiles.append(ht)

                for mh in range(MH):
                    w2t = w2p.tile([128, F], bf16)
                    for fo in range(4):
                        nc.sync.dma_start(
                            w2t[:, ts(fo, 896)], w2s[pi, mh, :, ts(fo, 896)]
                        )
                    for coff, cw in chunks:
                        psy = psyp.tile([128, NT], f32)
                        for kf in range(MF):
                            nc.tensor.matmul(
                                psy[:, :cw],
                                w2t[:, ts(kf, 128)],
                                h_tiles[kf][:, coff : coff + cw],
                                start=(kf == 0),
                                stop=(kf == MF - 1),
                            )
                        yt = yp.tile([128, NT], f32)
                        nc.vector.tensor_copy(yt[:, :cw], psy[:, :cw])
                        h0 = cw // 2
                        nc.sync.dma_start(
                            yT_r[mh, :, off + coff : off + coff + h0],
                            yt[:, :h0],
                        )
                        nc.sync.dma_start(
                            yT_r[mh, :, off + coff + h0 : off + coff + cw],
                            yt[:, h0:cw],
                        )
                off += width

    nc.compile()
    return nc


def _route(x, gate_w, gate_b):
    """Host router: top-2 expert ids + renormalized combine weights."""
    logits = x.astype(np.float32) @ gate_w.astype(np.float32).T + gate_b.astype(
        np.float32
    )
    # top-2 by prob == top-2 by logit (softmax is monotonic); stable sort
    # matches jax.lax.top_k's lower-index-first tie-breaking.
    top2 = np.argsort(-logits, axis=-1, kind="stable")[:, :2]
    l2 = np.take_along_axis(logits, top2, axis=1)
    e2 = np.exp(l2 - l2.max(axis=1, keepdims=True))
    wts = e2 / e2.sum(axis=1, keepdims=True)
    return top2, wts.astype(np.float32)


def kernel(x, gate_w, gate_b, w1, w3, w2):
    import ml_dtypes
    from concourse.bass_utils import run_bass_kernel_spmd

    bf16 = ml_dtypes.bfloat16
    x = np.asarray(x, dtype=np.float32)
    T = x.shape[0]
    top2, wts = _route(x, np.asarray(gate_w), np.asarray(gate_b))

    idx_list, scale_list = [], []
    for e in range(E):
        sel = top2 == e                      # [T, 2] bool
        tok = np.nonzero(sel.any(axis=1))[0]
        idx_list.append(tok)
        # each token picks an expert at most once, so this take is unique
        which = sel[tok, 1].astype(np.int64)  # 0 if slot0, 1 if slot1
        scale_list.append(wts[tok, which])

    cnts = [len(i) for i in idx_list]
    plan = _balanced_plan(cnts)
    if plan is None:
        plan = _uniform_plan(max(cnts))
    widths, bins = plan
    P = len(widths)
    C = sum(widths)

    nc = _compile_cache.get(widths)
    if nc is None:
        nc = _build(widths)
        _compile_cache[widths] = nc

    # split each expert's token list over its bins, in (pass, core) order
    offs = np.cumsum([0] + list(widths))
    bin_tok = {}    # (core, pass) -> token index array
    used = [0] * E
    for p in range(P):
        for ci in range(E):
            e = bins[p][ci]
            take = min(widths[p], cnts[e] - used[e])
            bin_tok[(ci, p)] = idx_list[e][used[e] : used[e] + take]
            used[e] += take
    assert all(used[e] == cnts[e] for e in range(E)), (used, cnts)

    # per-expert bf16 weight blocks in matmul tile layout
    # W[k*128+p, m*128+c] -> [m, p, k*128+c]: 2KB-contiguous lhsT tiles
    w1 = np.asarray(w1, dtype=np.float32)
    w3 = np.asarray(w3, dtype=np.float32)
    w2 = np.asarray(w2, dtype=np.float32)
    w1b = [
        np.ascontiguousarray(
            w1[e].reshape(KH, 128, MF, 128).transpose(2, 1, 0, 3).reshape(MF, 128, H)
        ).astype(bf16)
        for e in range(E)
    ]
    w3b = [
        np.ascontiguousarray(
            w3[e].reshape(KH, 128, MF, 128).transpose(2, 1, 0, 3).reshape(MF, 128, H)
        ).astype(bf16)
        for e in range(E)
    ]
    w2b = [
        np.ascontiguousarray(
            w2[e].reshape(MF, 128, MH, 128).transpose(2, 1, 0, 3).reshape(MH, 128, F)
        ).astype(bf16)
        for e in range(E)
    ]

    xbf = x.astype(bf16)
    in_maps = []
    for ci in range(E):
        xTe = np.zeros((H, C), bf16)
        for p in range(P):
            tok = bin_tok[(ci, p)]
            if len(tok):
                xTe[:, offs[p] : offs[p] + len(tok)] = xbf[tok].T
        in_maps.append(
            {
                "xT": xTe,
                "w1s": np.stack([w1b[bins[p][ci]] for p in range(P)]),
                "w3s": np.stack([w3b[bins[p][ci]] for p in range(P)]),
                "w2s": np.stack([w2b[bins[p][ci]] for p in range(P)]),
            }
        )

    global _last_result
    res = run_bass_kernel_spmd(nc, in_maps, core_ids=list(range(E)))
    _last_result = res

    out = np.zeros((T, H), np.float32)
    for ci in range(E):
        yTe = res.results[ci]["yT"]
        for p in range(P):
            tok = bin_tok[(ci, p)]
            if len(tok) == 0:
                continue
            e = bins[p][ci]
            pos = np.searchsorted(idx_list[e], tok)
            out[tok] += yTe[:, offs[p] : offs[p] + len(tok)].T * scale_list[e][
                pos, None
            ]
    return out
